# revision 12
# baseline (speedup 1.0000x reference)
"""Bucket (block-diagonal) attention layer for Trainium2, 8 NeuronCores SPMD.

Sharding: data-parallel over batch (4) x tensor-parallel over head groups (2).
Core c = b*2 + g handles batch b, global heads [g*8, g*8+8).

Per-core math (local out dim 512 = 8 heads x 64):
  qT[dl, t] = sum_k Wq[g*512+dl, k] * x[b, t, k]  (+ bq)   [transposed layout]
  kT[dl, t] = likewise (bk dropped: constant-per-row score shifts cancel in
              softmax -- only bq enters scores via bq . k_j)
  v[t, dl]  = natural layout (bf16), with a ones-column appended per head so
              the attended matmul also produces the softmax denominator.
  scoresT[kt, qt] = matmul(lhsT=kT_head, rhs=qT_head)      (K=64)
  expT = exp(scoresT) in bf16 (no max subtraction; logits sigma ~3.3)
  att[qt, 0:64], den[qt] = matmul(lhsT=expT, rhs=[v_head | ones])  (bf16)
  y = att / den + (x_slice + bv)   [residual + bv folded on host, fp16]

Perf structure vs v1 baseline:
 - all attention matmuls 16-bit (v1 ran them fp32 = 4 cycles/row on PE)
 - scores for 4 heads share one PSUM bank -> one batched EXP per [128,512]
 - attended for 4 heads share one bank -> batched reciprocal + strided
   broadcast normalize on DVE (v1: per-head ops)
 - projections of quarter q+1 are emission-interleaved with attention of
   quarter q so the PE stays dense (HAM stays warm) and softmax latency
   hides under projection matmuls.
"""

import json
import sys

import numpy as np

FP16 = np.float16

B, S, D = 4, 4096, 1024
H, NB = 16, 32
HG = 2            # head groups (tensor parallel over heads)
NCORES = B * HG   # 8
DL = D // HG      # 512 local output dims per core
HL = H // HG      # 8 local heads
HD = D // H       # 64 head dim
BS = S // NB      # 128 bucket size
KC = D // 128     # 8 contraction chunks
NQ = 4            # token quarters processed as pipeline phases
TOKQ = S // NQ    # 1024 tokens per quarter
NBQ = TOKQ // BS  # 8 buckets per quarter
OD = DL // 128    # 4 out-dim partition tiles for qT/kT
VW = 66           # per-head block width in v tiles: 64 data + 1 ones + 1 pad

_built = None     # cached (nc,) so repeated kernel() calls reuse the program


def _apply_waitfix():
    """This container's walrus accepts at most ONE sem wait per instruction.
    Post-process the BIR json: hoist extra waits onto injected wait-only
    EventSemaphore instructions just before the owning instruction."""
    import concourse.bass as bass

    if getattr(bass.Bass, "_waitfix_applied", False):
        return
    orig = bass.Bass.to_json_bytes

    def _split(m):
        n = 0
        for f in m["functions"]:
            for blk in f["blocks"]:
                out = []
                for inst in blk["instructions"]:
                    si = inst.get("sync_info")
                    if si and si.get("on_wait") and len(si["on_wait"]) > 1:
                        waits = si["on_wait"]
                        si["on_wait"] = waits[-1:]
                        for k, w in enumerate(waits[:-1]):
                            out.append({
                                "debug": inst.get("debug", 0),
                                "engine": inst["engine"],
                                "ins": [],
                                "outs": [],
                                "name": f"wfix{n}_{k}_{inst['name']}",
                                "opcode": "EventSemaphore",
                                "sync_info": {"on_update": [], "on_wait": [w]},
                            })
                        n += 1
                    out.append(inst)
                blk["instructions"] = out
        return n

    def patched(self):
        m = json.loads(orig(self))
        _split(m)
        return json.dumps(m).encode()

    bass.Bass.to_json_bytes = patched
    bass.Bass._waitfix_applied = True


def _build():
    global _built
    if _built is not None:
        return _built

    _apply_waitfix()
    import concourse.bass as bass
    import concourse.tile as tile
    from concourse import mybir
    from concourse.bass import ts

    f32 = mybir.dt.float32
    fp16 = mybir.dt.float16
    bf16 = mybir.dt.bfloat16
    Act = mybir.ActivationFunctionType
    Alu = mybir.AluOpType

    nc = bass.Bass()
    xt = nc.dram_tensor("xt", [D, S], fp16, kind="ExternalInput")
    wq = nc.dram_tensor("wq", [D, DL], fp16, kind="ExternalInput")
    wk = nc.dram_tensor("wk", [D, DL], fp16, kind="ExternalInput")
    wv = nc.dram_tensor("wv", [D, DL], fp16, kind="ExternalInput")
    bqt = nc.dram_tensor("bq", [128, OD], f32, kind="ExternalInput")
    xres = nc.dram_tensor("xres", [S, DL], fp16, kind="ExternalInput")
    y = nc.dram_tensor("y", [S, DL], f32, kind="ExternalOutput")

    with tile.TileContext(nc) as tc:
        with (
            tc.tile_pool(name="wpool", bufs=1) as wpool,
            tc.tile_pool(name="xtp", bufs=2) as xtp,
            tc.tile_pool(name="qtp", bufs=2 * OD) as qtp,
            tc.tile_pool(name="ktp", bufs=2 * OD) as ktp,
            tc.tile_pool(name="vp", bufs=2 * NBQ) as vpool,
            tc.tile_pool(name="ep", bufs=4) as epool,
            tc.tile_pool(name="yp", bufs=3) as ypool,
            tc.tile_pool(name="xrp", bufs=4) as xrpool,
            tc.tile_pool(name="rp", bufs=8) as rpool,
            tc.tile_pool(name="ps_p", bufs=2, space="PSUM") as ps_p,
            tc.tile_pool(name="ps_s", bufs=4, space="PSUM") as ps_s,
            tc.tile_pool(name="ps_a", bufs=2, space="PSUM") as ps_a,
        ):
            # --- stationary weights + bias, one big DMA per tensor ---
            # (each dma_start costs ~300ns of descriptor fan-out on the sync
            # sequencer; 25 small weight loads serialized the head)
            def wload(src, nm, split=False):
                t = wpool.tile([128, KC, DL], fp16, tag=nm, name=nm)
                ap = src[:, :].rearrange("(kk p) d -> p kk d", p=128)
                if split:  # od-0 slice alone so the first LDW starts early
                    nc.sync.dma_start(out=t[:, :, 0:128], in_=ap[:, :, 0:128])
                    nc.sync.dma_start(out=t[:, :, 128:DL],
                                      in_=ap[:, :, 128:DL])
                else:
                    nc.sync.dma_start(out=t[:], in_=ap)
                return t

            state = {}  # per-quarter tiles: qt, kt, v

            def load_xt(q, tt=None, kk_split=False):
                """One big DMA per (quarter, token-half)."""
                tok0 = q * TOKQ
                if q not in state:
                    state[q] = {"xt": xtp.tile(
                        [128, KC, TOKQ], fp16, tag="xt", name="xt")}
                t = state[q]["xt"]
                for tth in (range(2) if tt is None else [tt]):
                    sl = slice(tok0 + tth * 512, tok0 + (tth + 1) * 512)
                    ap = xt[:, sl].rearrange("(kk p) t -> p kk t", p=128)
                    cs = slice(tth * 512, (tth + 1) * 512)
                    if kk_split:
                        nc.sync.dma_start(out=t[:, 0:4, cs], in_=ap[:, 0:4, :])
                        nc.sync.dma_start(out=t[:, 4:KC, cs],
                                          in_=ap[:, 4:KC, :])
                    else:
                        nc.sync.dma_start(out=t[:, :, cs], in_=ap)

            wq_t = wload(wq, "wq", split=True)
            load_xt(0, tt=0, kk_split=True)
            wk_t = wload(wk, "wk")
            load_xt(0, tt=1)
            bq_sb = wpool.tile([128, OD], f32, tag="bq")
            nc.sync.dma_start(out=bq_sb[:], in_=bqt[:, :])
            wv_t = wload(wv, "wv")
            wq_sb = [wq_t[:, kk, :] for kk in range(KC)]
            wk_sb = [wk_t[:, kk, :] for kk in range(KC)]
            wv_sb = [wv_t[:, kk, :] for kk in range(KC)]

            def proj_units(q):
                """Yield 24 emission units: 16 q/k groups + 8 v groups."""
                st = state[q]
                xt_t = st["xt"]
                qt_sb = [qtp.tile([128, TOKQ], fp16, tag="qt", name="qt")
                         for _ in range(OD)]
                kt_sb = [ktp.tile([128, TOKQ], fp16, tag="kt", name="kt")
                         for _ in range(OD)]
                v_sb = [vpool.tile([128, HL * VW], bf16, tag="v", name="v")
                        for _ in range(NBQ)]
                st["qt"], st["kt"], st["v"] = qt_sb, kt_sb, v_sb

                def qk_unit(which, od, tt):
                    def emit():
                        w_t = wq_t if which == "q" else wk_t
                        dst = qt_sb if which == "q" else kt_sb
                        p = ps_p.tile([128, 512], f32, tag="pp", name="pp")
                        for kk in range(KC):
                            nc.tensor.matmul(
                                p[:], w_t[:, kk, ts(od, 128)],
                                xt_t[:, kk, ts(tt, 512)],
                                start=(kk == 0), stop=(kk == KC - 1))
                        if which == "q":
                            nc.scalar.activation(
                                dst[od][:, ts(tt, 512)], p[:], Act.Identity,
                                bias=bq_sb[:, od:od + 1], scale=1.0)
                        else:
                            nc.scalar.copy(dst[od][:, ts(tt, 512)], p[:])
                    return emit

                def v_unit(vt):
                    def emit():
                        p = ps_p.tile([128, 512], f32, tag="pp", name="pp")
                        for kk in range(KC):
                            nc.tensor.matmul(
                                p[:], xt_t[:, kk, ts(vt, 128)], wv_t[:, kk, :],
                                start=(kk == 0), stop=(kk == KC - 1))
                        vt_sb = v_sb[vt]
                        v3 = vt_sb[:].rearrange("p (h c) -> p h c", c=VW)
                        nc.vector.memset(v3[:, :, 64:66], 1.0)
                        nc.vector.tensor_copy(
                            v3[:, :, 0:64],
                            p[:].rearrange("p (h c) -> p h c", c=HD))
                    return emit

                # tt-major so quarter 0 can start on the first half of xt;
                # q before k so the wk DMA hides under the q-unit stream
                units = []
                for tt in range(2):
                    for od in range(OD):
                        units.append(qk_unit("q", od, tt))
                    for od in range(OD):
                        units.append(qk_unit("k", od, tt))
                for vt in range(NBQ):
                    units.append(v_unit(vt))
                return units

            def attn_scores(q, bk):
                """Part 1: scores matmuls + batched EXP for one bucket."""
                st = state[q]
                qt_sb, kt_sb = st["qt"], st["kt"]
                col = ts(bk, BS)  # token slice within quarter
                se = ps_s.tile([128, 512], f32, tag="ps", name="ps_e")
                so = ps_s.tile([128, 512], f32, tag="ps", name="ps_o")
                # even heads first so the EXP of bank se can start while the
                # odd-head score matmuls still stream
                for h in (0, 2, 4, 6, 1, 3, 5, 7):
                    od, po = h // 2, (h % 2) * 64
                    bank = se if h % 2 == 0 else so
                    nc.tensor.matmul(
                        bank[:, ts(h // 2, 128)],
                        kt_sb[od][po:po + 64, col],
                        qt_sb[od][po:po + 64, col],
                        start=True, stop=True)
                ex_e = epool.tile([128, 512], bf16, tag="ex", name="ex_e")
                ex_o = epool.tile([128, 512], bf16, tag="ex", name="ex_o")
                nc.scalar.activation(ex_e[:], se[:], Act.Exp)
                nc.scalar.activation(ex_o[:], so[:], Act.Exp)
                st.setdefault("ex", {})[bk] = (ex_e, ex_o)

            def attn_out(q, bk):
                """Part 2: attended matmuls + normalize + residual + out."""
                st = state[q]
                v_sb = st["v"]
                ex_e, ex_o = st["ex"].pop(bk)
                tok0 = q * TOKQ
                xr = xrpool.tile([128, DL], fp16, tag="xres")
                nc.sync.dma_start(
                    out=xr[:],
                    in_=xres[tok0 + bk * BS:tok0 + (bk + 1) * BS, :])
                pe = ps_a.tile([128, HL // 2 * VW], f32, tag="pa", name="pa_e")
                po_ = ps_a.tile([128, HL // 2 * VW], f32, tag="pa", name="pa_o")
                for h in (0, 2, 4, 6, 1, 3, 5, 7):
                    ex = ex_e if h % 2 == 0 else ex_o
                    bank = pe if h % 2 == 0 else po_
                    slot = h // 2
                    nc.tensor.matmul(
                        bank[:, slot * VW:slot * VW + VW],
                        ex[:, ts(slot, 128)],
                        v_sb[bk][:, h * VW:(h + 1) * VW],
                        start=True, stop=True)
                yt = ypool.tile([128, DL], f32, tag="yt")
                for par, bank in ((0, pe), (1, po_)):
                    pav = bank[:].rearrange("p (h c) -> p h c", c=VW)
                    rc = rpool.tile([128, HL // 2], f32, tag="rc")
                    nc.vector.reciprocal(
                        rc[:].unsqueeze(2), pav[:, :, 64:65])
                    ytv = yt[:].rearrange(
                        "p (h two c) -> p h two c", two=2, c=HD)[:, :, par, :]
                    rcb = rc[:].unsqueeze(2).broadcast_to((128, HL // 2, HD))
                    nc.vector.tensor_tensor(
                        out=ytv, in0=pav[:, :, 0:HD], in1=rcb, op=Alu.mult)
                nc.vector.tensor_tensor(
                    out=yt[:], in0=yt[:], in1=xr[:], op=Alu.add)
                nc.sync.dma_start(
                    out=y[tok0 + bk * BS:tok0 + (bk + 1) * BS, :], in_=yt[:])

            # --- emission: per quarter, 16 q/k units then for each bucket
            # [v-unit, scores, attended(bk-1)] -- the EXP latency of bucket
            # bk hides under the v projection of bucket bk+1.  The last
            # bucket's attended spills into the next quarter's first unit.
            pending = None
            for q in range(NQ):
                units = proj_units(q)
                for i in range(2 * OD * 2):
                    units[i]()
                    if i == 0 and pending is not None:
                        attn_out(*pending)
                        pending = None
                    if i == 7 and q + 1 < NQ:
                        load_xt(q + 1)
                for bk in range(NBQ):
                    units[16 + bk]()
                    attn_scores(q, bk)
                    if bk > 0:
                        attn_out(q, bk - 1)
                pending = (q, NBQ - 1)
            attn_out(*pending)

    _built = nc
    return nc


def _prep_in_maps(x, Wq, bq, Wk, bk, Wv, bv):
    x = np.asarray(x, np.float32)
    Wq = np.asarray(Wq, np.float32)
    Wv = np.asarray(Wv, np.float32)
    Wk = np.asarray(Wk, np.float32)
    bq = np.asarray(bq, np.float32)
    bv = np.asarray(bv, np.float32)

    xt_b = [np.ascontiguousarray(x[b].T).astype(FP16) for b in range(B)]
    wq_g, wk_g, wv_g, bq_g = [], [], [], []
    for g in range(HG):
        sl = slice(g * DL, (g + 1) * DL)
        wq_g.append(np.ascontiguousarray(Wq[sl, :].T).astype(FP16))
        wk_g.append(np.ascontiguousarray(Wk[sl, :].T).astype(FP16))
        wv_g.append(np.ascontiguousarray(Wv[sl, :].T).astype(FP16))
        bq_g.append(np.ascontiguousarray(
            bq[sl].reshape(DL // 128, 128).T).astype(np.float32))

    in_maps = []
    for c in range(NCORES):
        b, g = c // HG, c % HG
        sl = slice(g * DL, (g + 1) * DL)
        xres = (x[b][:, sl] + bv[None, sl]).astype(FP16)
        in_maps.append({
            "xt": xt_b[b], "wq": wq_g[g], "wk": wk_g[g], "wv": wv_g[g],
            "bq": bq_g[g], "xres": np.ascontiguousarray(xres),
        })
    return in_maps


def _gather(results):
    out = np.empty((B, S, D), np.float32)
    for c, r in enumerate(results):
        b, g = c // HG, c % HG
        out[b, :, g * DL:(g + 1) * DL] = r["y"]
    return out


def _run(inputs, trace=False, trace_cores=None):
    nc = _build()
    from concourse.bass_utils import run_bass_kernel_spmd

    in_maps = _prep_in_maps(**inputs)
    res = run_bass_kernel_spmd(
        nc, in_maps, core_ids=list(range(NCORES)), trace=trace,
        trace_cores=trace_cores)
    return _gather(res.results), res


def kernel(**inputs):
    out, _ = _run(inputs, trace=False)
    return out


def kernel_traced(trace_cores=None, **inputs):
    """For test.py: returns (output, BassKernelResults with exec_time_ns)."""
    import types
    import trn_agent_boot.trn_boot as tb

    if "antenv.axon_hooks" not in sys.modules:
        hooks = types.ModuleType("antenv.axon_hooks")
        state = [None]
        hooks.set_axon_ntff_profile_hook = lambda h: state.__setitem__(0, h)
        hooks.get_axon_ntff_profile_hook = lambda: state[0]
        sys.modules["antenv.axon_hooks"] = hooks
        hooks.set_axon_ntff_profile_hook(
            tb._ntff_profile_via_ctypes("/opt/axon/libaxon_pjrt.so"))
    return _run(inputs, trace=True, trace_cores=trace_cores)


# revision 19
# speedup vs baseline: 1.0055x; 1.0055x over previous
"""Bucket (block-diagonal) attention layer for Trainium2, 8 NeuronCores SPMD.

Sharding: data-parallel over batch (4) x tensor-parallel over head groups (2).
Core c = b*2 + g handles batch b, global heads [g*8, g*8+8).

Per-core math (local out dim 512 = 8 heads x 64):
  qT[dl, t] = sum_k Wq[g*512+dl, k] * x[b, t, k]  (+ bq)   [transposed layout]
  kT[dl, t] = likewise (bk dropped: constant-per-row score shifts cancel in
              softmax -- only bq enters scores via bq . k_j)
  v[t, dl]  = natural layout (bf16), with a ones-column appended per head so
              the attended matmul also produces the softmax denominator.
  scoresT[kt, qt] = matmul(lhsT=kT_head, rhs=qT_head)      (K=64)
  expT = exp(scoresT) in bf16 (no max subtraction; logits sigma ~3.3)
  att[qt, 0:64], den[qt] = matmul(lhsT=expT, rhs=[v_head | ones])  (bf16)
  y = att / den + (x_slice + bv)   [residual + bv folded on host, fp16]

Perf structure vs v1 baseline:
 - all attention matmuls 16-bit (v1 ran them fp32 = 4 cycles/row on PE)
 - scores for 4 heads share one PSUM bank -> one batched EXP per [128,512]
 - attended for 4 heads share one bank -> batched reciprocal + strided
   broadcast normalize on DVE (v1: per-head ops)
 - projections of quarter q+1 are emission-interleaved with attention of
   quarter q so the PE stays dense (HAM stays warm) and softmax latency
   hides under projection matmuls.
"""

import json
import sys

import numpy as np

FP16 = np.float16

B, S, D = 4, 4096, 1024
H, NB = 16, 32
HG = 2            # head groups (tensor parallel over heads)
NCORES = B * HG   # 8
DL = D // HG      # 512 local output dims per core
HL = H // HG      # 8 local heads
HD = D // H       # 64 head dim
BS = S // NB      # 128 bucket size
KC = D // 128     # 8 contraction chunks
NQ = 4            # token quarters processed as pipeline phases
TOKQ = S // NQ    # 1024 tokens per quarter
NBQ = TOKQ // BS  # 8 buckets per quarter
OD = DL // 128    # 4 out-dim partition tiles for qT/kT
VW = 66           # per-head block width in v tiles: 64 data + 1 ones + 1 pad

_built = None     # cached (nc,) so repeated kernel() calls reuse the program


def _apply_waitfix():
    """This container's walrus accepts at most ONE sem wait per instruction.
    Post-process the BIR json: hoist extra waits onto injected wait-only
    EventSemaphore instructions just before the owning instruction."""
    import concourse.bass as bass

    if getattr(bass.Bass, "_waitfix_applied", False):
        return
    orig = bass.Bass.to_json_bytes

    def _split(m):
        n = 0
        for f in m["functions"]:
            for blk in f["blocks"]:
                out = []
                for inst in blk["instructions"]:
                    si = inst.get("sync_info")
                    if si and si.get("on_wait") and len(si["on_wait"]) > 1:
                        waits = si["on_wait"]
                        si["on_wait"] = waits[-1:]
                        for k, w in enumerate(waits[:-1]):
                            out.append({
                                "debug": inst.get("debug", 0),
                                "engine": inst["engine"],
                                "ins": [],
                                "outs": [],
                                "name": f"wfix{n}_{k}_{inst['name']}",
                                "opcode": "EventSemaphore",
                                "sync_info": {"on_update": [], "on_wait": [w]},
                            })
                        n += 1
                    out.append(inst)
                blk["instructions"] = out
        return n

    def patched(self):
        m = json.loads(orig(self))
        _split(m)
        return json.dumps(m).encode()

    bass.Bass.to_json_bytes = patched
    bass.Bass._waitfix_applied = True


def _build():
    global _built
    if _built is not None:
        return _built

    _apply_waitfix()
    import concourse.bass as bass
    import concourse.tile as tile
    from concourse import mybir
    from concourse.bass import ts

    f32 = mybir.dt.float32
    fp16 = mybir.dt.float16
    bf16 = mybir.dt.bfloat16
    Act = mybir.ActivationFunctionType
    Alu = mybir.AluOpType

    # All inputs are host-side pre-arranged partition-major so every DMA
    # reads large contiguous spans per partition (small descriptors make
    # the DMA queues descriptor-rate-bound: ~1KB/descr was the v6 head).
    nc = bass.Bass()
    xt = nc.dram_tensor("xt", [128, NQ * KC * TOKQ], fp16,
                        kind="ExternalInput")
    wq = nc.dram_tensor("wq", [128, KC * DL], fp16, kind="ExternalInput")
    wk = nc.dram_tensor("wk", [128, KC * DL], fp16, kind="ExternalInput")
    wv = nc.dram_tensor("wv", [128, KC * DL], fp16, kind="ExternalInput")
    bqt = nc.dram_tensor("bq", [128, OD], f32, kind="ExternalInput")
    xres = nc.dram_tensor("xres", [128, NB * DL], fp16, kind="ExternalInput")
    y = nc.dram_tensor("y", [S, DL], f32, kind="ExternalOutput")

    with tile.TileContext(nc) as tc:
        with (
            tc.tile_pool(name="wpool", bufs=1) as wpool,
            tc.tile_pool(name="xtp", bufs=2) as xtp,
            tc.tile_pool(name="qtp", bufs=2 * OD) as qtp,
            tc.tile_pool(name="ktp", bufs=2 * OD) as ktp,
            tc.tile_pool(name="vp", bufs=2 * NBQ) as vpool,
            tc.tile_pool(name="ep", bufs=4) as epool,
            tc.tile_pool(name="yp", bufs=3) as ypool,
            tc.tile_pool(name="xrp", bufs=2) as xrpool,
            tc.tile_pool(name="rp", bufs=8) as rpool,
            tc.tile_pool(name="ps_p", bufs=2, space="PSUM") as ps_p,
            tc.tile_pool(name="ps_s", bufs=4, space="PSUM") as ps_s,
            tc.tile_pool(name="ps_a", bufs=2, space="PSUM") as ps_a,
        ):
            # --- stationary weights + bias, one big DMA per tensor ---
            # (each dma_start costs ~300ns of descriptor fan-out on the sync
            # sequencer; 25 small weight loads serialized the head)
            # --- PE warm-up: dummy matmuls on zeros during the DMA head so
            # the HAM clock-gate releases (1.2->2.4GHz) before real work ---
            warm = wpool.tile([128, 640], fp16, tag="warm")
            nc.vector.memset(warm[:], 0.0)
            pwarm = ps_s.tile([128, 512], f32, tag="ps", name="pwarm")
            for i in range(16):
                nc.tensor.matmul(pwarm[:], warm[:, 512:640], warm[:, 0:512],
                                 start=(i == 0), stop=(i == 15))
            wsink = wpool.tile([128, 1], f32, tag="wsink")
            nc.vector.reciprocal(wsink[:], pwarm[:, 0:1])

            def wload(src, nm, split=False):
                t = wpool.tile([128, KC, DL], fp16, tag=nm, name=nm)
                ap = src[:, :].rearrange("p (kk d) -> p kk d", kk=KC)
                if split:  # kk halves: first matmuls start on half the data
                    nc.sync.dma_start(out=t[:, 0:4, :], in_=ap[:, 0:4, :])
                    nc.sync.dma_start(out=t[:, 4:KC, :], in_=ap[:, 4:KC, :])
                else:
                    nc.sync.dma_start(out=t[:], in_=ap)
                return t

            state = {}  # per-quarter tiles: xt, xr, qt, kt, v, ex

            xt4 = xt[:, :].rearrange("p (q kk t) -> p q kk t", q=NQ, kk=KC)
            xr3 = xres[:, :].rearrange("p (nb d) -> p nb d", d=DL)

            def load_xt(q, kk_split=False):
                if q not in state:
                    state[q] = {"xt": xtp.tile(
                        [128, KC, TOKQ], fp16, tag="xt", name="xt")}
                t = state[q]["xt"]
                if kk_split:
                    nc.sync.dma_start(out=t[:, 0:4, :], in_=xt4[:, q, 0:4, :])
                    nc.sync.dma_start(out=t[:, 4:KC, :],
                                      in_=xt4[:, q, 4:KC, :])
                else:
                    nc.sync.dma_start(out=t[:], in_=xt4[:, q, :, :])

            def load_xr(q):
                t = xrpool.tile([128, NBQ, DL], fp16, tag="xres", name="xres")
                nc.sync.dma_start(
                    out=t[:], in_=xr3[:, q * NBQ:(q + 1) * NBQ, :])
                state[q]["xr"] = t

            wq_t = wload(wq, "wq", split=True)
            load_xt(0, kk_split=True)
            wk_t = wload(wk, "wk")
            bq_sb = wpool.tile([128, OD], f32, tag="bq")
            nc.sync.dma_start(out=bq_sb[:], in_=bqt[:, :])
            wv_t = wload(wv, "wv")
            load_xr(0)
            wq_sb = [wq_t[:, kk, :] for kk in range(KC)]
            wk_sb = [wk_t[:, kk, :] for kk in range(KC)]
            wv_sb = [wv_t[:, kk, :] for kk in range(KC)]

            def proj_units(q):
                """Yield 24 emission units: 16 q/k groups + 8 v groups."""
                st = state[q]
                xt_t = st["xt"]
                qt_sb = [qtp.tile([128, TOKQ], fp16, tag="qt", name="qt")
                         for _ in range(OD)]
                kt_sb = [ktp.tile([128, TOKQ], fp16, tag="kt", name="kt")
                         for _ in range(OD)]
                v_sb = [vpool.tile([128, HL * VW], bf16, tag="v", name="v")
                        for _ in range(NBQ)]
                st["qt"], st["kt"], st["v"] = qt_sb, kt_sb, v_sb

                def qk_unit(which, od, tt):
                    def emit():
                        w_t = wq_t if which == "q" else wk_t
                        dst = qt_sb if which == "q" else kt_sb
                        p = ps_p.tile([128, 512], f32, tag="pp", name="pp")
                        for kk in range(KC):
                            nc.tensor.matmul(
                                p[:], w_t[:, kk, ts(od, 128)],
                                xt_t[:, kk, ts(tt, 512)],
                                start=(kk == 0), stop=(kk == KC - 1))
                        if which == "q":
                            nc.scalar.activation(
                                dst[od][:, ts(tt, 512)], p[:], Act.Identity,
                                bias=bq_sb[:, od:od + 1], scale=1.0)
                        else:
                            nc.scalar.copy(dst[od][:, ts(tt, 512)], p[:])
                    return emit

                def v_unit(vt):
                    def emit():
                        p = ps_p.tile([128, 512], f32, tag="pp", name="pp")
                        for kk in range(KC):
                            nc.tensor.matmul(
                                p[:], xt_t[:, kk, ts(vt, 128)], wv_t[:, kk, :],
                                start=(kk == 0), stop=(kk == KC - 1))
                        vt_sb = v_sb[vt]
                        v3 = vt_sb[:].rearrange("p (h c) -> p h c", c=VW)
                        nc.vector.memset(v3[:, :, 64:66], 1.0)
                        nc.vector.tensor_copy(
                            v3[:, :, 0:64],
                            p[:].rearrange("p (h c) -> p h c", c=HD))
                    return emit

                # tt-major so quarter 0 can start on the first half of xt;
                # q before k so the wk DMA hides under the q-unit stream
                units = []
                for tt in range(2):
                    for od in range(OD):
                        units.append(qk_unit("q", od, tt))
                    for od in range(OD):
                        units.append(qk_unit("k", od, tt))
                for vt in range(NBQ):
                    units.append(v_unit(vt))
                return units

            def attn_scores(q, bk):
                """Part 1: scores matmuls + batched EXP for one bucket."""
                st = state[q]
                qt_sb, kt_sb = st["qt"], st["kt"]
                col = ts(bk, BS)  # token slice within quarter
                se = ps_s.tile([128, 512], f32, tag="ps", name="ps_e")
                so = ps_s.tile([128, 512], f32, tag="ps", name="ps_o")
                # even heads first so the EXP of bank se can start while the
                # odd-head score matmuls still stream
                for h in (0, 2, 4, 6, 1, 3, 5, 7):
                    od, po = h // 2, (h % 2) * 64
                    bank = se if h % 2 == 0 else so
                    nc.tensor.matmul(
                        bank[:, ts(h // 2, 128)],
                        kt_sb[od][po:po + 64, col],
                        qt_sb[od][po:po + 64, col],
                        start=True, stop=True)
                ex_e = epool.tile([128, 512], bf16, tag="ex", name="ex_e")
                ex_o = epool.tile([128, 512], bf16, tag="ex", name="ex_o")
                nc.scalar.activation(ex_e[:], se[:], Act.Exp)
                nc.scalar.activation(ex_o[:], so[:], Act.Exp)
                st.setdefault("ex", {})[bk] = (ex_e, ex_o)

            def attn_out(q, bk):
                """Part 2: attended matmuls + normalize + residual + out."""
                st = state[q]
                v_sb = st["v"]
                ex_e, ex_o = st["ex"].pop(bk)
                tok0 = q * TOKQ
                xr = st["xr"][:, bk, :]
                pe = ps_a.tile([128, HL // 2 * VW], f32, tag="pa", name="pa_e")
                po_ = ps_a.tile([128, HL // 2 * VW], f32, tag="pa", name="pa_o")
                for h in (0, 2, 4, 6, 1, 3, 5, 7):
                    ex = ex_e if h % 2 == 0 else ex_o
                    bank = pe if h % 2 == 0 else po_
                    slot = h // 2
                    nc.tensor.matmul(
                        bank[:, slot * VW:slot * VW + VW],
                        ex[:, ts(slot, 128)],
                        v_sb[bk][:, h * VW:(h + 1) * VW],
                        start=True, stop=True)
                yt = ypool.tile([128, DL], f32, tag="yt")
                for par, bank in ((0, pe), (1, po_)):
                    pav = bank[:].rearrange("p (h c) -> p h c", c=VW)
                    rc = rpool.tile([128, HL // 2], f32, tag="rc")
                    nc.vector.reciprocal(
                        rc[:].unsqueeze(2), pav[:, :, 64:65])
                    ytv = yt[:].rearrange(
                        "p (h two c) -> p h two c", two=2, c=HD)[:, :, par, :]
                    rcb = rc[:].unsqueeze(2).broadcast_to((128, HL // 2, HD))
                    nc.vector.tensor_tensor(
                        out=ytv, in0=pav[:, :, 0:HD], in1=rcb, op=Alu.mult)
                nc.vector.tensor_tensor(
                    out=yt[:], in0=yt[:], in1=xr[:], op=Alu.add)
                nc.sync.dma_start(
                    out=y[tok0 + bk * BS:tok0 + (bk + 1) * BS, :], in_=yt[:])

            # --- emission: per quarter, 16 q/k units then for each bucket
            # [v-unit, scores, attended(bk-1)] -- the EXP latency of bucket
            # bk hides under the v projection of bucket bk+1.  The last
            # bucket's attended spills into the next quarter's first unit.
            pending = None
            for q in range(NQ):
                units = proj_units(q)
                for i in range(2 * OD * 2):
                    units[i]()
                    if i == 0 and pending is not None:
                        attn_out(*pending)
                        pending = None
                    if i == 7 and q + 1 < NQ:
                        load_xt(q + 1)
                        load_xr(q + 1)
                for bk in range(NBQ):
                    units[16 + bk]()
                    attn_scores(q, bk)
                    if bk > 0:
                        attn_out(q, bk - 1)
                pending = (q, NBQ - 1)
            attn_out(*pending)

    _built = nc
    return nc


def _prep_in_maps(x, Wq, bq, Wk, bk, Wv, bv):
    x = np.asarray(x, np.float32)
    Wq = np.asarray(Wq, np.float32)
    Wv = np.asarray(Wv, np.float32)
    Wk = np.asarray(Wk, np.float32)
    bq = np.asarray(bq, np.float32)
    bv = np.asarray(bv, np.float32)

    # partition-major layouts (see dram_tensor comments in _build):
    #   xt  [128, NQ, KC, TOKQ]: [p, q, kk, t] = x.T[kk*128+p, q*TOKQ+t]
    #   w*  [128, KC, DL]:       [p, kk, d]    = W.T[kk*128+p, d]
    #   xres[128, NB, DL]:       [p, nb, d]    = x[nb*128+p, d] + bv[d]
    def _xt_layout(a):  # a: [D, S]
        return np.ascontiguousarray(
            a.reshape(KC, 128, NQ, TOKQ).transpose(1, 2, 0, 3)
        ).reshape(128, NQ * KC * TOKQ)

    def _w_layout(a):  # a: [D, DL]
        return np.ascontiguousarray(
            a.reshape(KC, 128, DL).transpose(1, 0, 2)).reshape(128, KC * DL)

    xt_b = [_xt_layout(x[b].T.astype(FP16)) for b in range(B)]
    wq_g, wk_g, wv_g, bq_g = [], [], [], []
    for g in range(HG):
        sl = slice(g * DL, (g + 1) * DL)
        wq_g.append(_w_layout(Wq[sl, :].T.astype(FP16)))
        wk_g.append(_w_layout(Wk[sl, :].T.astype(FP16)))
        wv_g.append(_w_layout(Wv[sl, :].T.astype(FP16)))
        bq_g.append(np.ascontiguousarray(
            bq[sl].reshape(DL // 128, 128).T).astype(np.float32))

    in_maps = []
    for c in range(NCORES):
        b, g = c // HG, c % HG
        sl = slice(g * DL, (g + 1) * DL)
        xres = (x[b][:, sl] + bv[None, sl]).astype(FP16)  # [S, DL]
        xres = np.ascontiguousarray(
            xres.reshape(NB, 128, DL).transpose(1, 0, 2)
        ).reshape(128, NB * DL)
        in_maps.append({
            "xt": xt_b[b], "wq": wq_g[g], "wk": wk_g[g], "wv": wv_g[g],
            "bq": bq_g[g], "xres": xres,
        })
    return in_maps


def _gather(results):
    out = np.empty((B, S, D), np.float32)
    for c, r in enumerate(results):
        b, g = c // HG, c % HG
        out[b, :, g * DL:(g + 1) * DL] = r["y"]
    return out


def _run(inputs, trace=False, trace_cores=None):
    nc = _build()
    from concourse.bass_utils import run_bass_kernel_spmd

    in_maps = _prep_in_maps(**inputs)
    res = run_bass_kernel_spmd(
        nc, in_maps, core_ids=list(range(NCORES)), trace=trace,
        trace_cores=trace_cores)
    return _gather(res.results), res


def kernel(**inputs):
    out, _ = _run(inputs, trace=False)
    return out


def kernel_traced(trace_cores=None, **inputs):
    """For test.py: returns (output, BassKernelResults with exec_time_ns)."""
    import types
    import trn_agent_boot.trn_boot as tb

    if "antenv.axon_hooks" not in sys.modules:
        hooks = types.ModuleType("antenv.axon_hooks")
        state = [None]
        hooks.set_axon_ntff_profile_hook = lambda h: state.__setitem__(0, h)
        hooks.get_axon_ntff_profile_hook = lambda: state[0]
        sys.modules["antenv.axon_hooks"] = hooks
        hooks.set_axon_ntff_profile_hook(
            tb._ntff_profile_via_ctypes("/opt/axon/libaxon_pjrt.so"))
    return _run(inputs, trace=True, trace_cores=trace_cores)
